# revision 13
# baseline (speedup 1.0000x reference)
"""Trainium2 Bass kernel for the bilevel logit-assignment flow problem.

Reference math (N=384, cutoff-2 paths):
    A = (adj > 0) & ~eye
    E = A * exp(-lam * dist)                       # "edge weight" matrix
    Z = E + offdiag(E @ E)                          # softmax denominator
    W = demand / Z    (demand = od offdiag; od > 0 and Z > 0 off-diag
                       for this input family; diag handled by eps + od=0)
    flows = W*E + E*(W @ E^T) + E*(E^T @ W)

Sharding with node-relabeling: the computation is equivariant under a
symmetric permutation of nodes, so core i receives all matrices rolled
by -48*i on both axes. Its origin slice is then ALWAYS rows 0..47,
making Es a free view of E (identical SPMD program on all cores), and
its `rows` flow contribution lands in p3 tile 0, partitions 0..47 —
merged into the p3 output on-device. Host un-rolls the outputs and sums.

Device-side structure:
    E tile  = exp(lam*(big*adj - dist) - BIG)       # STT(DVE) + Exp(Act)
    ET tile = same, from host-transposed adj/dist
    Z       = EEs psum, seeded with identity@Es (adds Es on the PE)
    zinv    = reciprocal_approx_fast(Z + 1e-30)     # 1 DVE op, ~51 ULP
    W       = od ⊙ zinv
    T2 psum = identity@W + W @ E^T  (seed trick again: rows add is free)
    p3      = E ⊙ (Es^T @ W);  p3[tile0, 0:48] += Es ⊙ T2
Outputs ship as f16 (host accumulates in f32).
"""

import numpy as np

import concourse.bass as bass
import concourse.mybir as mybir
import concourse.tile as tile
from concourse import bacc
from concourse.bass_utils import run_bass_kernel_spmd
from concourse.masks import make_identity

N = 384
NCORES = 8
S = N // NCORES  # 48 origins per core
P = 128
NT = N // P  # 3 partition tiles

F32 = mybir.dt.float32
F32R = mybir.dt.float32r
F16 = mybir.dt.float16
U8 = mybir.dt.uint8
Act = mybir.ActivationFunctionType
Alu = mybir.AluOpType

BIG = 160.0  # exp(-BIG) == +0.0 in fp32 (no denormal residue)


def build_program(lam: float) -> bass.Bass:
    nc = bacc.Bacc(
        "TRN2",
        target_bir_lowering=False,
        debug=False,
        num_devices=NCORES,
        enable_asserts=False,
    )

    def mm(ap):
        return ap.bitcast(F32R)

    big = BIG / lam  # el = adj*big - dist;  E = exp(lam*el - BIG)

    # partition-tiled layouts: [p, t, n] == full[128*t + p, n] (rolled space)
    adj8 = nc.dram_tensor("adj8", [P, NT, N], U8, kind="ExternalInput")
    dsth = nc.dram_tensor("dsth", [P, NT, N], F16, kind="ExternalInput")
    adjT8 = nc.dram_tensor("adjT8", [P, NT, N], U8, kind="ExternalInput")
    dstTh = nc.dram_tensor("dstTh", [P, NT, N], F16, kind="ExternalInput")
    odt = nc.dram_tensor("odt", [S, N], F32, kind="ExternalInput")
    p3 = nc.dram_tensor("p3_t", [P, NT, N], F16, kind="ExternalOutput")

    with tile.TileContext(nc) as tc:
        with (
            tc.tile_pool(name="sb", bufs=1) as sb,
            tc.tile_pool(name="pst", bufs=2, space="PSUM") as pst,
            tc.tile_pool(name="psacc", bufs=1, space="PSUM") as psacc,
            tc.tile_pool(name="psp3", bufs=2, space="PSUM") as psp3,
        ):
            b8 = sb.tile([P, NT, N], U8)
            dh = sb.tile([P, NT, N], F16)
            bT8 = sb.tile([P, NT, N], U8)
            dhT = sb.tile([P, NT, N], F16)
            ods = sb.tile([S, N], F32)

            # ---- input DMA issue (scalar gets the early-needed tensors) ----
            nc.scalar.dma_start(b8[:], adj8[:])
            nc.scalar.dma_start(dh[:], dsth[:])
            nc.scalar.dma_start(ods[:], odt[:])
            nc.sync.dma_start(bT8[:], adjT8[:])
            nc.sync.dma_start(dhT[:], dstTh[:])

            ident = sb.tile([P, P], F32)
            make_identity(nc, ident[:])
            identm = sb.tile([S, S], F32)
            nc.vector.tensor_copy(mm(identm[:]), ident[:S, :S])
            nbig = sb.tile([P, 1], F32)
            nc.gpsimd.memset(nbig[:], -BIG)

            # ---- E build ----
            el = sb.tile([P, NT, N], F32)
            E = sb.tile([P, NT, N], F32)
            for t in range(NT):
                nc.vector.scalar_tensor_tensor(
                    el[:, t, :], b8[:, t, :], big, dh[:, t, :], Alu.mult, Alu.subtract
                )
                nc.scalar.activation(
                    mm(E[:, t, :]), el[:, t, :], Act.Exp, bias=nbig[:], scale=lam
                )
            Es = E[0:S, 0, :]  # origin slice == rows 0..47 in rolled space

            # ---- EsT via PE transposes; EEs = Es + Es @ E (identity-seeded) ----
            EsT = sb.tile([P, NT, S], F32)
            for c in range(NT):
                tp = pst.tile([P, S], F32, tag="tp")
                nc.tensor.transpose(
                    mm(tp[:]), mm(Es[:, P * c : P * (c + 1)]), mm(identm[:])
                )
                nc.vector.tensor_copy(mm(EsT[:, c, :]), tp[:])
            EEs = psacc.tile([S, N], F32, tag="EEs")
            nc.tensor.matmul(EEs[:], mm(identm[:]), mm(Es), start=True, stop=False)
            for t in range(NT):
                nc.tensor.matmul(
                    EEs[:], mm(EsT[:, t, :]), mm(E[:, t, :]),
                    start=False, stop=(t == NT - 1),
                )

            # ---- ET direct build from transposed inputs ----
            etl = sb.tile([P, NT, N], F32)
            ET = sb.tile([P, NT, N], F32)
            for t in range(NT):
                nc.vector.scalar_tensor_tensor(
                    etl[:, t, :], bT8[:, t, :], big, dhT[:, t, :],
                    Alu.mult, Alu.subtract,
                )
                nc.scalar.activation(
                    mm(ET[:, t, :]), etl[:, t, :], Act.Exp, bias=nbig[:], scale=lam
                )

            # ---- W = od ⊙ recip(EEs + eps) ----
            zsafe = sb.tile([S, N], F32)
            zinv = sb.tile([S, N], F32)
            W = sb.tile([S, N], F32)
            nc.scalar.activation(zsafe[:], EEs[:], Act.Copy, bias=1e-30, scale=1.0)
            nc.vector.reciprocal_approx_fast(zinv[:], zsafe[:])
            nc.vector.tensor_mul(mm(W[:]), ods[:], zinv[:])

            # ---- T2 = W + W @ E^T (identity-seeded accumulation) ----
            WsT = sb.tile([P, NT, S], F32)
            T2 = psacc.tile([S, N], F32, tag="T2")
            nc.tensor.matmul(T2[:], mm(identm[:]), mm(W[:]), start=True, stop=False)
            for c in range(NT):
                tpw = pst.tile([P, S], F32, tag="tp")
                nc.tensor.transpose(
                    mm(tpw[:]), mm(W[:, P * c : P * (c + 1)]), mm(identm[:])
                )
                nc.vector.tensor_copy(mm(WsT[:, c, :]), tpw[:])
                nc.tensor.matmul(
                    T2[:], mm(WsT[:, c, :]), mm(ET[:, c, :]),
                    start=False, stop=(c == NT - 1),
                )

            # ---- P3 = Es^T @ W per row-tile; p3 = E ⊙ P3 (f16 out) ----
            outs = []
            for mt in range(NT):
                P3 = psp3.tile([P, N], F32, tag="P3")
                nc.tensor.matmul(
                    P3[:], mm(Es[:, P * mt : P * (mt + 1)]), mm(W[:]),
                    start=True, stop=True,
                )
                out_t = sb.tile([P, N], F16)
                nc.vector.tensor_mul(out_t[:], E[:, mt, :], P3[:])
                outs.append(out_t)
                if mt == 1:
                    nc.scalar.dma_start(p3[:, 1, :], out_t[:])
                elif mt == 2:
                    nc.sync.dma_start(p3[:, 2, :], out_t[:])

            # ---- rows = Es ⊙ T2, merged into p3 tile 0 (partitions 0..47) ----
            rows_sb = sb.tile([S, N], F32)
            nc.vector.tensor_mul(rows_sb[:], Es, T2[:])
            nc.vector.tensor_add(outs[0][0:S, :], outs[0][0:S, :], rows_sb[:])
            nc.sync.dma_start(p3[:, 0, :], outs[0][:])

    nc.compile()
    return nc


_PROGRAM_CACHE: dict = {}


def _get_program(lam: float) -> bass.Bass:
    if lam not in _PROGRAM_CACHE:
        _PROGRAM_CACHE[lam] = build_program(lam)
    return _PROGRAM_CACHE[lam]


def _tile_rows(x: np.ndarray) -> np.ndarray:
    """[384, N] row-major -> [128, 3, N] partition-tiled layout."""
    return np.ascontiguousarray(x.reshape(NT, P, -1).transpose(1, 0, 2))


def _untile_rows(x: np.ndarray) -> np.ndarray:
    """[128, 3, N] partition-tiled -> [384, N]."""
    return x.transpose(1, 0, 2).reshape(N, -1)


def make_in_maps(od, adj, dist):
    adjz = adj.astype(np.uint8)
    np.fill_diagonal(adjz, 0)
    odz = od.copy()
    np.fill_diagonal(odz, 0.0)
    disth = dist.astype(np.float16)
    in_maps = []
    for i in range(NCORES):
        r = S * i
        a = np.roll(adjz, (-r, -r), axis=(0, 1))
        d = np.roll(disth, (-r, -r), axis=(0, 1))
        ods = np.roll(odz, (-r, -r), axis=(0, 1))[:S]
        in_maps.append(
            {
                "adj8": _tile_rows(a),
                "dsth": _tile_rows(d),
                "adjT8": _tile_rows(np.ascontiguousarray(a.T)),
                "dstTh": _tile_rows(np.ascontiguousarray(d.T)),
                "odt": np.ascontiguousarray(ods),
            }
        )
    return in_maps


def gather(results) -> np.ndarray:
    out = np.zeros((N, N), np.float32)
    for i in range(NCORES):
        r = S * i
        p3f = _untile_rows(results[i]["p3_t"]).astype(np.float32)
        out += np.roll(p3f, (r, r), axis=(0, 1))
    return out


def kernel(od, adj, dist, lambda_param, capacity=None, **_unused) -> np.ndarray:
    od = np.ascontiguousarray(np.asarray(od, dtype=np.float32))
    adj = np.ascontiguousarray(np.asarray(adj, dtype=np.int32))
    dist = np.ascontiguousarray(np.asarray(dist, dtype=np.float32))
    lam = float(np.asarray(lambda_param))
    nc = _get_program(lam)
    res = run_bass_kernel_spmd(nc, make_in_maps(od, adj, dist), list(range(NCORES)))
    return gather(res.results)


# revision 14
# speedup vs baseline: 1.1082x; 1.1082x over previous
"""Trainium2 Bass kernel for the bilevel logit-assignment flow problem.

Reference math (N=384, cutoff-2 paths):
    A = (adj > 0) & ~eye
    E = A * exp(-lam * dist)                       # "edge weight" matrix
    Z = E + offdiag(E @ E)                          # softmax denominator
    W = demand / Z    (demand = od offdiag; od > 0 and Z > 0 off-diag
                       for this input family; diag handled by eps + od=0)
    flows = W*E + E*(W @ E^T) + E*(E^T @ W)

Sharding with node-relabeling: the computation is equivariant under a
symmetric permutation of nodes, so core i receives all matrices rolled
by -48*i on both axes. Its origin slice is then ALWAYS rows 0..47,
making Es a free view of E (identical SPMD program on all cores), and
its `rows` flow contribution lands in p3 tile 0, partitions 0..47 —
merged into the p3 output on-device. Host un-rolls the outputs and sums.

Device-side structure:
    E tile  = exp(lam*(big*adj - dist) - BIG)       # STT(DVE) + Exp(Act)
    ET tile = same, from host-transposed adj/dist
    Z       = EEs psum, seeded with identity@Es (adds Es on the PE)
    zinv    = reciprocal_approx_fast(Z + 1e-30)     # 1 DVE op, ~51 ULP
    W       = od ⊙ zinv
    T2 psum = identity@W + W @ E^T  (seed trick again: rows add is free)
    p3      = E ⊙ (Es^T @ W);  p3[tile0, 0:48] += Es ⊙ T2
Outputs ship as f16 (host accumulates in f32).
"""

import numpy as np

import concourse.bass as bass
import concourse.mybir as mybir
import concourse.tile as tile
from concourse import bacc
from concourse.bass_utils import run_bass_kernel_spmd
from concourse.masks import make_identity

N = 384
NCORES = 8
S = N // NCORES  # 48 origins per core
P = 128
NT = N // P  # 3 partition tiles

F32 = mybir.dt.float32
F32R = mybir.dt.float32r
F16 = mybir.dt.float16
U8 = mybir.dt.uint8
Act = mybir.ActivationFunctionType
Alu = mybir.AluOpType

BIG = 160.0  # exp(-BIG) == +0.0 in fp32 (no denormal residue)


def build_program(lam: float) -> bass.Bass:
    nc = bacc.Bacc(
        "TRN2",
        target_bir_lowering=False,
        debug=False,
        num_devices=NCORES,
        enable_asserts=False,
    )

    def mm(ap):
        return ap.bitcast(F32R)

    big = BIG / lam  # el = adj*big - dist;  E = exp(lam*el - BIG)

    # partition-tiled layouts: [p, t, n] == full[128*t + p, n] (rolled space)
    adj8 = nc.dram_tensor("adj8", [P, NT, N], U8, kind="ExternalInput")
    dsth = nc.dram_tensor("dsth", [P, NT, N], F16, kind="ExternalInput")
    adjT8 = nc.dram_tensor("adjT8", [P, NT, N], U8, kind="ExternalInput")
    dstTh = nc.dram_tensor("dstTh", [P, NT, N], F16, kind="ExternalInput")
    odt = nc.dram_tensor("odt", [S, N], F32, kind="ExternalInput")
    p3 = nc.dram_tensor("p3_t", [P, NT, N], F16, kind="ExternalOutput")

    with tile.TileContext(nc) as tc:
        with (
            tc.tile_pool(name="sb", bufs=1) as sb,
            tc.tile_pool(name="pst", bufs=3, space="PSUM") as pst,
            tc.tile_pool(name="psacc", bufs=1, space="PSUM") as psacc,
            tc.tile_pool(name="psp3", bufs=3, space="PSUM") as psp3,
        ):
            b8 = sb.tile([P, NT, N], U8)
            dh = sb.tile([P, NT, N], F16)
            bT8 = sb.tile([P, NT, N], U8)
            dhT = sb.tile([P, NT, N], F16)
            ods = sb.tile([S, N], F32)

            # ---- input DMA issue (E-build inputs ring doorbells first) ----
            nc.scalar.dma_start(dh[:], dsth[:])
            nc.scalar.dma_start(b8[:], adj8[:])
            nc.sync.dma_start(ods[:], odt[:])
            nc.sync.dma_start(bT8[:], adjT8[:])
            nc.sync.dma_start(dhT[:], dstTh[:])

            ident = sb.tile([P, P], F32)
            make_identity(nc, ident[:])
            identm = sb.tile([S, S], F32)
            nc.vector.tensor_copy(mm(identm[:]), ident[:S, :S])
            nbig = sb.tile([P, 1], F32)
            nc.gpsimd.memset(nbig[:], -BIG)

            # ---- E build ----
            el = sb.tile([P, NT, N], F32)
            E = sb.tile([P, NT, N], F32)
            for t in range(NT):
                nc.vector.scalar_tensor_tensor(
                    el[:, t, :], b8[:, t, :], big, dh[:, t, :], Alu.mult, Alu.subtract
                )
                nc.scalar.activation(
                    mm(E[:, t, :]), el[:, t, :], Act.Exp, bias=nbig[:], scale=lam
                )
            Es = E[0:S, 0, :]  # origin slice == rows 0..47 in rolled space

            # ---- ET direct build from transposed inputs ----
            # ET[p, u, k] = E[k, 128u+p]; its first S free-columns are EsT:
            # ET[:, u, 0:S][k, o] = E[o, 128u+k] = Es^T tile u, for free.
            etl = sb.tile([P, NT, N], F32)
            ET = sb.tile([P, NT, N], F32)
            for t in range(NT):
                nc.vector.scalar_tensor_tensor(
                    etl[:, t, :], bT8[:, t, :], big, dhT[:, t, :],
                    Alu.mult, Alu.subtract,
                )
                nc.scalar.activation(
                    mm(ET[:, t, :]), etl[:, t, :], Act.Exp, bias=nbig[:], scale=lam
                )

            # ---- EEs = Es + Es @ E (identity-seeded; lhsT = ET slices) ----
            EEs = psacc.tile([S, N], F32, tag="EEs")
            nc.tensor.matmul(EEs[:], mm(identm[:]), mm(Es), start=True, stop=False)
            for t in range(NT):
                nc.tensor.matmul(
                    EEs[:], mm(ET[:, t, 0:S]), mm(E[:, t, :]),
                    start=False, stop=(t == NT - 1),
                )

            # ---- W = od ⊙ recip(EEs + eps) ----
            zsafe = sb.tile([S, N], F32)
            zinv = sb.tile([S, N], F32)
            W = sb.tile([S, N], F32)
            nc.scalar.activation(zsafe[:], EEs[:], Act.Copy, bias=1e-30, scale=1.0)
            nc.vector.reciprocal_approx_fast(zinv[:], zsafe[:])
            nc.vector.tensor_mul(mm(W[:]), ods[:], zinv[:])

            # ---- T2 = W + W @ E^T (identity-seeded accumulation) ----
            WsT = sb.tile([P, NT, S], F32)
            T2 = psacc.tile([S, N], F32, tag="T2")
            nc.tensor.matmul(T2[:], mm(identm[:]), mm(W[:]), start=True, stop=False)
            for c in range(NT):
                tpw = pst.tile([P, S], F32, tag="tp")
                nc.tensor.transpose(
                    mm(tpw[:]), mm(W[:, P * c : P * (c + 1)]), mm(identm[:])
                )
                nc.vector.tensor_copy(mm(WsT[:, c, :]), tpw[:])
                nc.tensor.matmul(
                    T2[:], mm(WsT[:, c, :]), mm(ET[:, c, :]),
                    start=False, stop=(c == NT - 1),
                )

            # ---- P3 = Es^T @ W per row-tile; p3 = E ⊙ P3 (f16 out) ----
            out_t0 = sb.tile([P, N], F16)
            out_t1 = sb.tile([P, N], F16)
            out_t2 = sb.tile([P, N], F16)
            outs = [out_t0, out_t1, out_t2]
            for mt in range(NT):
                P3 = psp3.tile([P, N], F32, tag="P3")
                nc.tensor.matmul(
                    P3[:], mm(Es[:, P * mt : P * (mt + 1)]), mm(W[:]),
                    start=True, stop=True,
                )
                nc.vector.tensor_mul(outs[mt][:], E[:, mt, :], P3[:])
                if mt == 1:
                    nc.scalar.dma_start(p3[:, 1, :], outs[1][:])
                elif mt == 2:
                    nc.sync.dma_start(p3[:, 2, :], outs[2][:])

            # ---- rows = Es ⊙ T2, merged into p3 tile 0 (partitions 0..47) ----
            rows_sb = sb.tile([S, N], F32)
            nc.vector.tensor_mul(rows_sb[:], Es, T2[:])
            nc.vector.tensor_add(out_t0[0:S, :], out_t0[0:S, :], rows_sb[:])
            nc.sync.dma_start(p3[:, 0, :], out_t0[:])

    nc.compile()
    return nc


_PROGRAM_CACHE: dict = {}


def _get_program(lam: float) -> bass.Bass:
    if lam not in _PROGRAM_CACHE:
        _PROGRAM_CACHE[lam] = build_program(lam)
    return _PROGRAM_CACHE[lam]


def _tile_rows(x: np.ndarray) -> np.ndarray:
    """[384, N] row-major -> [128, 3, N] partition-tiled layout."""
    return np.ascontiguousarray(x.reshape(NT, P, -1).transpose(1, 0, 2))


def _untile_rows(x: np.ndarray) -> np.ndarray:
    """[128, 3, N] partition-tiled -> [384, N]."""
    return x.transpose(1, 0, 2).reshape(N, -1)


def make_in_maps(od, adj, dist):
    adjz = adj.astype(np.uint8)
    np.fill_diagonal(adjz, 0)
    odz = od.copy()
    np.fill_diagonal(odz, 0.0)
    disth = dist.astype(np.float16)
    in_maps = []
    for i in range(NCORES):
        r = S * i
        a = np.roll(adjz, (-r, -r), axis=(0, 1))
        d = np.roll(disth, (-r, -r), axis=(0, 1))
        ods = np.roll(odz, (-r, -r), axis=(0, 1))[:S]
        in_maps.append(
            {
                "adj8": _tile_rows(a),
                "dsth": _tile_rows(d),
                "adjT8": _tile_rows(np.ascontiguousarray(a.T)),
                "dstTh": _tile_rows(np.ascontiguousarray(d.T)),
                "odt": np.ascontiguousarray(ods),
            }
        )
    return in_maps


def gather(results) -> np.ndarray:
    out = np.zeros((N, N), np.float32)
    for i in range(NCORES):
        r = S * i
        p3f = _untile_rows(results[i]["p3_t"]).astype(np.float32)
        out += np.roll(p3f, (r, r), axis=(0, 1))
    return out


def kernel(od, adj, dist, lambda_param, capacity=None, **_unused) -> np.ndarray:
    od = np.ascontiguousarray(np.asarray(od, dtype=np.float32))
    adj = np.ascontiguousarray(np.asarray(adj, dtype=np.int32))
    dist = np.ascontiguousarray(np.asarray(dist, dtype=np.float32))
    lam = float(np.asarray(lambda_param))
    nc = _get_program(lam)
    res = run_bass_kernel_spmd(nc, make_in_maps(od, adj, dist), list(range(NCORES)))
    return gather(res.results)


# revision 16
# speedup vs baseline: 1.2319x; 1.1115x over previous
"""Trainium2 Bass kernel for the bilevel logit-assignment flow problem.

Reference math (N=384, cutoff-2 paths):
    A = (adj > 0) & ~eye
    E = A * exp(-lam * dist)                       # "edge weight" matrix
    Z = E + offdiag(E @ E)                          # softmax denominator
    W = demand / Z    (demand = od offdiag; od > 0 and Z > 0 off-diag
                       for this input family; diag handled by eps + od=0)
    flows = W*E + E*(W @ E^T) + E*(E^T @ W)

Sharding with node-relabeling: the computation is equivariant under a
symmetric permutation of nodes, so core i receives all matrices rolled
by -48*i on both axes. Its origin slice is then ALWAYS rows 0..47,
making Es a free view of E (identical SPMD program on all cores), and
its `rows` flow contribution lands in p3 tile 0, partitions 0..47 —
merged into the p3 output on-device. Host un-rolls the outputs and sums.

Device-side structure:
    E tile  = exp(lam*(big*adj - dist) - BIG)       # STT(DVE) + Exp(Act)
    ET tile = same, from host-transposed adj/dist
    Z       = EEs psum, seeded with identity@Es (adds Es on the PE)
    zinv    = reciprocal_approx_fast(Z + 1e-30)     # 1 DVE op, ~51 ULP
    W       = od ⊙ zinv
    T2 psum = identity@W + W @ E^T  (seed trick again: rows add is free)
    p3      = E ⊙ (Es^T @ W);  p3[tile0, 0:48] += Es ⊙ T2
Outputs ship as f16 (host accumulates in f32).
"""

import numpy as np

import concourse.bass as bass
import concourse.mybir as mybir
import concourse.tile as tile
from concourse import bacc
from concourse.bass_utils import run_bass_kernel_spmd
from concourse.masks import make_identity

N = 384
NCORES = 8
S = N // NCORES  # 48 origins per core
P = 128
NT = N // P  # 3 partition tiles

F32 = mybir.dt.float32
F32R = mybir.dt.float32r
F16 = mybir.dt.float16
U8 = mybir.dt.uint8
I8 = mybir.dt.int8
Act = mybir.ActivationFunctionType
Alu = mybir.AluOpType

BIG = 160.0  # exp(-BIG) == +0.0 in fp32 (no denormal residue)


def build_program(lam: float) -> bass.Bass:
    nc = bacc.Bacc(
        "TRN2",
        target_bir_lowering=False,
        debug=False,
        num_devices=NCORES,
        enable_asserts=False,
    )

    def mm(ap):
        return ap.bitcast(F32R)

    big = BIG / lam  # el = adj*big - dist;  E = exp(lam*el - BIG)

    # partition-tiled layouts: [p, t, n] == full[128*t + p, n] (rolled space)
    adj8 = nc.dram_tensor("adj8", [P, NT, N], I8, kind="ExternalInput")
    dsth = nc.dram_tensor("dsth", [P, NT, N], F16, kind="ExternalInput")
    adjT8 = nc.dram_tensor("adjT8", [P, NT, N], I8, kind="ExternalInput")
    dstTh = nc.dram_tensor("dstTh", [P, NT, N], F16, kind="ExternalInput")
    odt = nc.dram_tensor("odt", [S, N], F32, kind="ExternalInput")
    p3 = nc.dram_tensor("p3_t", [P, NT, N], F16, kind="ExternalOutput")

    with tile.TileContext(nc) as tc:
        with (
            tc.tile_pool(name="sb", bufs=1) as sb,
            tc.tile_pool(name="pst", bufs=3, space="PSUM") as pst,
            tc.tile_pool(name="psacc", bufs=1, space="PSUM") as psacc,
            tc.tile_pool(name="psp3", bufs=1, space="PSUM") as psp3,
        ):
            b8 = sb.tile([P, NT, N], I8)
            dh = sb.tile([P, NT, N], F16)
            bT8 = sb.tile([P, NT, N], I8)
            dhT = sb.tile([P, NT, N], F16)
            ods = sb.tile([S, N], F32)

            # ---- input DMA issue (E-build inputs ring doorbells first) ----
            nc.scalar.dma_start(dh[:], dsth[:])
            nc.scalar.dma_start(b8[:], adj8[:])
            nc.sync.dma_start(ods[:], odt[:])
            nc.sync.dma_start(bT8[:], adjT8[:])
            nc.sync.dma_start(dhT[:], dstTh[:])

            ident = sb.tile([P, P], F32)
            make_identity(nc, ident[:])
            identm = sb.tile([S, S], F32)
            nc.vector.tensor_copy(mm(identm[:]), ident[:S, :S])

            # ---- E build ----
            el = sb.tile([P, NT, N], F32)
            E = sb.tile([P, NT, N], F32)
            for t in range(NT):
                nc.vector.scalar_tensor_tensor(
                    el[:, t, :], b8[:, t, :], big, dh[:, t, :], Alu.mult, Alu.subtract
                )
                nc.scalar.activation(
                    mm(E[:, t, :]), el[:, t, :], Act.Exp, scale=lam
                )
            Es = E[0:S, 0, :]  # origin slice == rows 0..47 in rolled space

            # ---- EsT via PE transposes; EEs = Es + Es @ E (identity-seeded) ----
            EsT = sb.tile([P, NT, S], F32)
            EEs = psacc.tile([S, N], F32, tag="EEs")
            nc.tensor.matmul(EEs[:], mm(identm[:]), mm(Es), start=True, stop=False)
            for t in range(NT):
                tp = pst.tile([P, S], F32, tag="tp")
                nc.tensor.transpose(
                    mm(tp[:]), mm(Es[:, P * t : P * (t + 1)]), mm(identm[:])
                )
                nc.vector.tensor_copy(mm(EsT[:, t, :]), tp[:])
                nc.tensor.matmul(
                    EEs[:], mm(EsT[:, t, :]), mm(E[:, t, :]),
                    start=False, stop=(t == NT - 1),
                )

            # ---- ET direct build from transposed inputs ----
            etl = sb.tile([P, NT, N], F32)
            ET = sb.tile([P, NT, N], F32)
            for t in range(NT):
                nc.vector.scalar_tensor_tensor(
                    etl[:, t, :], bT8[:, t, :], big, dhT[:, t, :],
                    Alu.mult, Alu.subtract,
                )
                nc.scalar.activation(
                    mm(ET[:, t, :]), etl[:, t, :], Act.Exp, scale=lam
                )

            # ---- W = od ⊙ recip(EEs + eps) ----
            zsafe = sb.tile([S, N], F32)
            zinv = sb.tile([S, N], F32)
            W = sb.tile([S, N], F32)
            nc.vector.tensor_single_scalar(zsafe[:], EEs[:], 1e-30, Alu.add)
            nc.vector.reciprocal_approx_fast(zinv[:], zsafe[:])
            nc.vector.tensor_mul(mm(W[:]), ods[:], zinv[:])

            # ---- P3(mt) = Es^T @ W; tile 0 also accumulates
            #      T2 = W + W @ E^T into partitions 0..47 (Es == E0[0:48],
            #      so p3_t0 = E0 ⊙ (P3 + pad(T2)) covers the rows terms) ----
            out_t0 = sb.tile([P, N], F16)
            out_t1 = sb.tile([P, N], F16)
            out_t2 = sb.tile([P, N], F16)
            WsT = sb.tile([P, NT, S], F32)
            P30 = psp3.tile([P, N], F32, tag="P30")
            P31 = psp3.tile([P, N], F32, tag="P31")
            P32 = psp3.tile([P, N], F32, tag="P32")
            nc.tensor.matmul(
                P30[:], mm(Es[:, 0:P]), mm(W[:]), start=True, stop=False
            )
            nc.tensor.matmul(
                P30[0:S, :], mm(identm[:]), mm(W[:]), start=False, stop=False
            )
            nc.tensor.matmul(
                P31[:], mm(Es[:, P : 2 * P]), mm(W[:]), start=True, stop=True
            )
            nc.vector.tensor_mul(out_t1[:], E[:, 1, :], P31[:])
            nc.scalar.dma_start(p3[:, 1, :], out_t1[:])
            nc.tensor.matmul(
                P32[:], mm(Es[:, 2 * P : N]), mm(W[:]), start=True, stop=True
            )
            nc.vector.tensor_mul(out_t2[:], E[:, 2, :], P32[:])
            nc.sync.dma_start(p3[:, 2, :], out_t2[:])
            for c in range(NT):
                tpw = pst.tile([P, S], F32, tag="tp")
                nc.tensor.transpose(
                    mm(tpw[:]), mm(W[:, P * c : P * (c + 1)]), mm(identm[:])
                )
                nc.vector.tensor_copy(mm(WsT[:, c, :]), tpw[:])
                nc.tensor.matmul(
                    P30[0:S, :], mm(WsT[:, c, :]), mm(ET[:, c, :]),
                    start=False, stop=(c == NT - 1),
                )
            nc.vector.tensor_mul(out_t0[:], E[:, 0, :], P30[:])
            nc.sync.dma_start(p3[:, 0, :], out_t0[:])

    nc.compile()
    return nc


_PROGRAM_CACHE: dict = {}


def _get_program(lam: float) -> bass.Bass:
    if lam not in _PROGRAM_CACHE:
        _PROGRAM_CACHE[lam] = build_program(lam)
    return _PROGRAM_CACHE[lam]


def _tile_rows(x: np.ndarray) -> np.ndarray:
    """[384, N] row-major -> [128, 3, N] partition-tiled layout."""
    return np.ascontiguousarray(x.reshape(NT, P, -1).transpose(1, 0, 2))


def _untile_rows(x: np.ndarray) -> np.ndarray:
    """[128, 3, N] partition-tiled -> [384, N]."""
    return x.transpose(1, 0, 2).reshape(N, -1)


def make_in_maps(od, adj, dist):
    adjz = adj.astype(np.int8)
    np.fill_diagonal(adjz, 0)
    adjz -= 1  # edge -> 0, non-edge -> -1: el = big*(adj-1) - dist
    odz = od.copy()
    np.fill_diagonal(odz, 0.0)
    disth = dist.astype(np.float16)
    in_maps = []
    for i in range(NCORES):
        r = S * i
        a = np.roll(adjz, (-r, -r), axis=(0, 1))
        d = np.roll(disth, (-r, -r), axis=(0, 1))
        ods = np.roll(odz, (-r, -r), axis=(0, 1))[:S]
        in_maps.append(
            {
                "adj8": _tile_rows(a),
                "dsth": _tile_rows(d),
                "adjT8": _tile_rows(np.ascontiguousarray(a.T)),
                "dstTh": _tile_rows(np.ascontiguousarray(d.T)),
                "odt": np.ascontiguousarray(ods),
            }
        )
    return in_maps


def gather(results) -> np.ndarray:
    out = np.zeros((N, N), np.float32)
    for i in range(NCORES):
        r = S * i
        p3f = _untile_rows(results[i]["p3_t"]).astype(np.float32)
        out += np.roll(p3f, (r, r), axis=(0, 1))
    return out


def kernel(od, adj, dist, lambda_param, capacity=None, **_unused) -> np.ndarray:
    od = np.ascontiguousarray(np.asarray(od, dtype=np.float32))
    adj = np.ascontiguousarray(np.asarray(adj, dtype=np.int32))
    dist = np.ascontiguousarray(np.asarray(dist, dtype=np.float32))
    lam = float(np.asarray(lambda_param))
    nc = _get_program(lam)
    res = run_bass_kernel_spmd(nc, make_in_maps(od, adj, dist), list(range(NCORES)))
    return gather(res.results)


# revision 17
# speedup vs baseline: 1.2998x; 1.0551x over previous
"""Trainium2 Bass kernel for the bilevel logit-assignment flow problem.

Reference math (N=384, cutoff-2 paths):
    A = (adj > 0) & ~eye
    E = A * exp(-lam * dist)                       # "edge weight" matrix
    Z = E + offdiag(E @ E)                          # softmax denominator
    W = demand / Z    (demand = od offdiag; od > 0 and Z > 0 off-diag
                       for this input family; diag handled by eps + od=0)
    flows = W*E + E*(W @ E^T) + E*(E^T @ W)

Sharding with node-relabeling: the computation is equivariant under a
symmetric permutation of nodes, so core i receives all matrices rolled
by -48*i on both axes. Its origin slice is then ALWAYS rows 0..47,
making Es a free view of E (identical SPMD program on all cores), and
its `rows` flow contribution lands in p3 tile 0, partitions 0..47 —
merged into the p3 output on-device. Host un-rolls the outputs and sums.

Device-side structure:
    E tile  = exp(lam*(big*adj - dist) - BIG)       # STT(DVE) + Exp(Act)
    ET tile = same, from host-transposed adj/dist
    Z       = EEs psum, seeded with identity@Es (adds Es on the PE)
    zinv    = reciprocal_approx_fast(Z + 1e-30)     # 1 DVE op, ~51 ULP
    W       = od ⊙ zinv
    T2 psum = identity@W + W @ E^T  (seed trick again: rows add is free)
    p3      = E ⊙ (Es^T @ W);  p3[tile0, 0:48] += Es ⊙ T2
Outputs ship as f16 (host accumulates in f32).
"""

import numpy as np

import concourse.bass as bass
import concourse.mybir as mybir
import concourse.tile as tile
from concourse import bacc
from concourse.bass_utils import run_bass_kernel_spmd
from concourse.masks import make_identity

N = 384
NCORES = 8
S = N // NCORES  # 48 origins per core
P = 128
NT = N // P  # 3 partition tiles

F32 = mybir.dt.float32
F32R = mybir.dt.float32r
F16 = mybir.dt.float16
U8 = mybir.dt.uint8
I8 = mybir.dt.int8
Act = mybir.ActivationFunctionType
Alu = mybir.AluOpType

BIG = 160.0  # exp(-BIG) == +0.0 in fp32 (no denormal residue)


def build_program(lam: float) -> bass.Bass:
    nc = bacc.Bacc(
        "TRN2",
        target_bir_lowering=False,
        debug=False,
        num_devices=NCORES,
        enable_asserts=False,
    )

    def mm(ap):
        return ap.bitcast(F32R)

    big = BIG / lam  # el = adj*big - dist;  E = exp(lam*el - BIG)

    # partition-tiled layouts: [p, t, n] == full[128*t + p, n] (rolled space)
    adj8 = nc.dram_tensor("adj8", [P, NT, N], I8, kind="ExternalInput")
    dsth = nc.dram_tensor("dsth", [P, NT, N], F16, kind="ExternalInput")
    adjT8 = nc.dram_tensor("adjT8", [P, NT, N], I8, kind="ExternalInput")
    dstTh = nc.dram_tensor("dstTh", [P, NT, N], F16, kind="ExternalInput")
    odt = nc.dram_tensor("odt", [S, N], F32, kind="ExternalInput")
    p3 = nc.dram_tensor("p3_t", [P, NT, N], F16, kind="ExternalOutput")

    with tile.TileContext(nc) as tc:
        with (
            tc.tile_pool(name="sb", bufs=1) as sb,
            tc.tile_pool(name="pst", bufs=3, space="PSUM") as pst,
            tc.tile_pool(name="psacc", bufs=1, space="PSUM") as psacc,
            tc.tile_pool(name="psp3", bufs=1, space="PSUM") as psp3,
        ):
            b8 = sb.tile([P, NT, N], I8)
            dh = sb.tile([P, NT, N], F16)
            bT8 = sb.tile([P, NT, N], I8)
            dhT = sb.tile([P, NT, N], F16)
            ods = sb.tile([S, N], F32)

            # ---- input DMA issue (E-build inputs ring doorbells first) ----
            nc.scalar.dma_start(dh[:, 0, :], dsth[:, 0, :])
            nc.scalar.dma_start(b8[:], adj8[:])
            nc.scalar.dma_start(dh[:, 1, :], dsth[:, 1, :])
            nc.scalar.dma_start(dh[:, 2, :], dsth[:, 2, :])
            nc.sync.dma_start(ods[:], odt[:])
            nc.sync.dma_start(bT8[:], adjT8[:])
            nc.sync.dma_start(dhT[:], dstTh[:])

            ident = sb.tile([P, P], F32)
            make_identity(nc, ident[:])
            identm = sb.tile([S, S], F32)
            nc.vector.tensor_copy(mm(identm[:]), ident[:S, :S])

            # ---- E build ----
            el = sb.tile([P, NT, N], F32)
            E = sb.tile([P, NT, N], F32)
            for t in range(NT):
                nc.vector.scalar_tensor_tensor(
                    el[:, t, :], b8[:, t, :], big, dh[:, t, :], Alu.mult, Alu.subtract
                )
                nc.scalar.activation(
                    mm(E[:, t, :]), el[:, t, :], Act.Exp, scale=lam
                )
            Es = E[0:S, 0, :]  # origin slice == rows 0..47 in rolled space

            # ---- EsT via PE transposes; EEs = eps + Es + Es @ E ----
            epsv = sb.tile([S, N], F32)
            nc.gpsimd.memset(epsv[:], 1e-30)
            epsm = sb.tile([S, N], F32)
            nc.gpsimd.tensor_copy(mm(epsm[:]), epsv[:])
            EsT = sb.tile([P, NT, S], F32)
            EEs = psacc.tile([S, N], F32, tag="EEs")
            nc.tensor.matmul(EEs[:], mm(identm[:]), mm(epsm[:]), start=True, stop=False)
            nc.tensor.matmul(EEs[:], mm(identm[:]), mm(Es), start=False, stop=False)
            for t in range(NT):
                tp = pst.tile([P, S], F32, tag="tp")
                nc.tensor.transpose(
                    mm(tp[:]), mm(Es[:, P * t : P * (t + 1)]), mm(identm[:])
                )
                nc.scalar.copy(mm(EsT[:, t, :]), tp[:])
                nc.tensor.matmul(
                    EEs[:], mm(EsT[:, t, :]), mm(E[:, t, :]),
                    start=False, stop=(t == NT - 1),
                )

            # ---- ET direct build from transposed inputs ----
            etl = sb.tile([P, NT, N], F32)
            ET = sb.tile([P, NT, N], F32)
            for t in range(NT):
                nc.vector.scalar_tensor_tensor(
                    etl[:, t, :], bT8[:, t, :], big, dhT[:, t, :],
                    Alu.mult, Alu.subtract,
                )
                nc.scalar.activation(
                    mm(ET[:, t, :]), etl[:, t, :], Act.Exp, scale=lam
                )

            # ---- W = od ⊙ recip(EEs)  (eps already inside the psum) ----
            zinv = sb.tile([S, N], F32)
            W = sb.tile([S, N], F32)
            nc.vector.reciprocal_approx_fast(zinv[:], EEs[:])
            nc.vector.tensor_mul(mm(W[:]), ods[:], zinv[:])

            # ---- P3(mt) = Es^T @ W; tile 0 also accumulates
            #      T2 = W + W @ E^T into partitions 0..47 (Es == E0[0:48],
            #      so p3_t0 = E0 ⊙ (P3 + pad(T2)) covers the rows terms) ----
            out_t0 = sb.tile([P, N], F16)
            out_t1 = sb.tile([P, N], F16)
            out_t2 = sb.tile([P, N], F16)
            WsT = sb.tile([P, NT, S], F32)
            P30 = psp3.tile([P, N], F32, tag="P30")
            P31 = psp3.tile([P, N], F32, tag="P31")
            P32 = psp3.tile([P, N], F32, tag="P32")
            nc.tensor.matmul(
                P30[:], mm(Es[:, 0:P]), mm(W[:]), start=True, stop=False
            )
            nc.tensor.matmul(
                P30[0:S, :], mm(identm[:]), mm(W[:]), start=False, stop=False
            )
            nc.tensor.matmul(
                P31[:], mm(Es[:, P : 2 * P]), mm(W[:]), start=True, stop=True
            )
            nc.vector.tensor_mul(out_t1[:], E[:, 1, :], P31[:])
            nc.scalar.dma_start(p3[:, 1, :], out_t1[:])
            nc.tensor.matmul(
                P32[:], mm(Es[:, 2 * P : N]), mm(W[:]), start=True, stop=True
            )
            nc.vector.tensor_mul(out_t2[:], E[:, 2, :], P32[:])
            nc.sync.dma_start(p3[:, 2, :], out_t2[:])
            for c in range(NT):
                tpw = pst.tile([P, S], F32, tag="tp")
                nc.tensor.transpose(
                    mm(tpw[:]), mm(W[:, P * c : P * (c + 1)]), mm(identm[:])
                )
                nc.scalar.copy(mm(WsT[:, c, :]), tpw[:])
                nc.tensor.matmul(
                    P30[0:S, :], mm(WsT[:, c, :]), mm(ET[:, c, :]),
                    start=False, stop=(c == NT - 1),
                )
            nc.vector.tensor_mul(out_t0[:], E[:, 0, :], P30[:])
            nc.sync.dma_start(p3[:, 0, :], out_t0[:])

    nc.compile()
    return nc


_PROGRAM_CACHE: dict = {}


def _get_program(lam: float) -> bass.Bass:
    if lam not in _PROGRAM_CACHE:
        _PROGRAM_CACHE[lam] = build_program(lam)
    return _PROGRAM_CACHE[lam]


def _tile_rows(x: np.ndarray) -> np.ndarray:
    """[384, N] row-major -> [128, 3, N] partition-tiled layout."""
    return np.ascontiguousarray(x.reshape(NT, P, -1).transpose(1, 0, 2))


def _untile_rows(x: np.ndarray) -> np.ndarray:
    """[128, 3, N] partition-tiled -> [384, N]."""
    return x.transpose(1, 0, 2).reshape(N, -1)


def make_in_maps(od, adj, dist):
    adjz = adj.astype(np.int8)
    np.fill_diagonal(adjz, 0)
    adjz -= 1  # edge -> 0, non-edge -> -1: el = big*(adj-1) - dist
    odz = od.copy()
    np.fill_diagonal(odz, 0.0)
    disth = dist.astype(np.float16)
    in_maps = []
    for i in range(NCORES):
        r = S * i
        a = np.roll(adjz, (-r, -r), axis=(0, 1))
        d = np.roll(disth, (-r, -r), axis=(0, 1))
        ods = np.roll(odz, (-r, -r), axis=(0, 1))[:S]
        in_maps.append(
            {
                "adj8": _tile_rows(a),
                "dsth": _tile_rows(d),
                "adjT8": _tile_rows(np.ascontiguousarray(a.T)),
                "dstTh": _tile_rows(np.ascontiguousarray(d.T)),
                "odt": np.ascontiguousarray(ods),
            }
        )
    return in_maps


def gather(results) -> np.ndarray:
    out = np.zeros((N, N), np.float32)
    for i in range(NCORES):
        r = S * i
        p3f = _untile_rows(results[i]["p3_t"]).astype(np.float32)
        out += np.roll(p3f, (r, r), axis=(0, 1))
    return out


def kernel(od, adj, dist, lambda_param, capacity=None, **_unused) -> np.ndarray:
    od = np.ascontiguousarray(np.asarray(od, dtype=np.float32))
    adj = np.ascontiguousarray(np.asarray(adj, dtype=np.int32))
    dist = np.ascontiguousarray(np.asarray(dist, dtype=np.float32))
    lam = float(np.asarray(lambda_param))
    nc = _get_program(lam)
    res = run_bass_kernel_spmd(nc, make_in_maps(od, adj, dist), list(range(NCORES)))
    return gather(res.results)


# revision 18
# speedup vs baseline: 1.3372x; 1.0288x over previous
"""Trainium2 Bass kernel for the bilevel logit-assignment flow problem.

Reference math (N=384, cutoff-2 paths):
    A = (adj > 0) & ~eye
    E = A * exp(-lam * dist)                       # "edge weight" matrix
    Z = E + offdiag(E @ E)                          # softmax denominator
    W = demand / Z    (demand = od offdiag; od > 0 and Z > 0 off-diag
                       for this input family; diag handled by eps + od=0)
    flows = W*E + E*(W @ E^T) + E*(E^T @ W)

Sharding with node-relabeling: the computation is equivariant under a
symmetric permutation of nodes, so core i receives all matrices rolled
by -48*i on both axes. Its origin slice is then ALWAYS rows 0..47,
making Es a free view of E (identical SPMD program on all cores), and
its `rows` flow contribution lands in p3 tile 0, partitions 0..47 —
merged into the p3 output on-device. Host un-rolls the outputs and sums.

Device-side structure:
    E tile  = exp(lam*(big*adj - dist) - BIG)       # STT(DVE) + Exp(Act)
    ET tile = same, from host-transposed adj/dist
    Z       = EEs psum, seeded with identity@Es (adds Es on the PE)
    zinv    = reciprocal_approx_fast(Z + 1e-30)     # 1 DVE op, ~51 ULP
    W       = od ⊙ zinv
    T2 psum = identity@W + W @ E^T  (seed trick again: rows add is free)
    p3      = E ⊙ (Es^T @ W);  p3[tile0, 0:48] += Es ⊙ T2
Outputs ship as f16 (host accumulates in f32).
"""

import numpy as np

import concourse.bass as bass
import concourse.mybir as mybir
import concourse.tile as tile
from concourse import bacc
from concourse.bass_utils import run_bass_kernel_spmd
from concourse.masks import make_identity

N = 384
NCORES = 8
S = N // NCORES  # 48 origins per core
P = 128
NT = N // P  # 3 partition tiles

F32 = mybir.dt.float32
F32R = mybir.dt.float32r
F16 = mybir.dt.float16
U8 = mybir.dt.uint8
I8 = mybir.dt.int8
Act = mybir.ActivationFunctionType
Alu = mybir.AluOpType

BIG = 160.0  # exp(-BIG) == +0.0 in fp32 (no denormal residue)


def build_program(lam: float) -> bass.Bass:
    nc = bacc.Bacc(
        "TRN2",
        target_bir_lowering=False,
        debug=False,
        num_devices=NCORES,
        enable_asserts=False,
    )

    def mm(ap):
        return ap.bitcast(F32R)

    big = BIG / lam  # el = adj*big - dist;  E = exp(lam*el - BIG)

    # partition-tiled layouts: [p, t, n] == full[128*t + p, n] (rolled space)
    adj8 = nc.dram_tensor("adj8", [P, NT, N], I8, kind="ExternalInput")
    dsth = nc.dram_tensor("dsth", [P, NT, N], F16, kind="ExternalInput")
    adjT8 = nc.dram_tensor("adjT8", [P, NT, N], I8, kind="ExternalInput")
    dstTh = nc.dram_tensor("dstTh", [P, NT, N], F16, kind="ExternalInput")
    odt = nc.dram_tensor("odt", [S, N], F32, kind="ExternalInput")
    p3 = nc.dram_tensor("p3_t", [P, NT, N], F16, kind="ExternalOutput")

    with tile.TileContext(nc) as tc:
        with (
            tc.tile_pool(name="sb", bufs=1) as sb,
            tc.tile_pool(name="pst", bufs=3, space="PSUM") as pst,
            tc.tile_pool(name="psacc", bufs=1, space="PSUM") as psacc,
            tc.tile_pool(name="psp3", bufs=1, space="PSUM") as psp3,
        ):
            b8 = sb.tile([P, NT, N], I8)
            dh = sb.tile([P, NT, N], F16)
            bT8 = sb.tile([P, NT, N], I8)
            dhT = sb.tile([P, NT, N], F16)
            ods = sb.tile([S, N], F32)

            # ---- input DMA issue: E-build inputs ring first (scalar+sync);
            #      ET inputs ride gpsimd AFTER its setup ops so their
            #      doorbells ring late and don't congest the queues ----
            nc.scalar.dma_start(dh[:, 0, :], dsth[:, 0, :])
            nc.scalar.dma_start(b8[:, 0, :], adj8[:, 0, :])
            nc.sync.dma_start(dh[:, 1, :], dsth[:, 1, :])
            nc.sync.dma_start(b8[:, 1, :], adj8[:, 1, :])
            nc.sync.dma_start(dh[:, 2, :], dsth[:, 2, :])
            nc.sync.dma_start(b8[:, 2, :], adj8[:, 2, :])
            nc.sync.dma_start(ods[:], odt[:])

            ident = sb.tile([P, P], F32)
            make_identity(nc, ident[:])
            identm = sb.tile([S, S], F32)
            nc.vector.tensor_copy(mm(identm[:]), ident[:S, :S])
            nc.gpsimd.dma_start(bT8[:], adjT8[:])
            nc.gpsimd.dma_start(dhT[:], dstTh[:])

            # ---- E build ----
            el = sb.tile([P, NT, N], F32)
            E = sb.tile([P, NT, N], F32)
            for t in range(NT):
                nc.vector.scalar_tensor_tensor(
                    el[:, t, :], b8[:, t, :], big, dh[:, t, :], Alu.mult, Alu.subtract
                )
                nc.scalar.activation(
                    mm(E[:, t, :]), el[:, t, :], Act.Exp, scale=lam
                )
            Es = E[0:S, 0, :]  # origin slice == rows 0..47 in rolled space

            # ---- EsT via PE transposes; EEs = eps + Es + Es @ E ----
            epsv = sb.tile([S, N], F32)
            nc.gpsimd.memset(epsv[:], 1e-30)
            epsm = sb.tile([S, N], F32)
            nc.gpsimd.tensor_copy(mm(epsm[:]), epsv[:])
            EsT = sb.tile([P, NT, S], F32)
            EEs = psacc.tile([S, N], F32, tag="EEs")
            nc.tensor.matmul(EEs[:], mm(identm[:]), mm(epsm[:]), start=True, stop=False)
            nc.tensor.matmul(EEs[:], mm(identm[:]), mm(Es), start=False, stop=False)
            for t in range(NT):
                tp = pst.tile([P, S], F32, tag="tp")
                nc.tensor.transpose(
                    mm(tp[:]), mm(Es[:, P * t : P * (t + 1)]), mm(identm[:])
                )
                nc.scalar.copy(mm(EsT[:, t, :]), tp[:])
                nc.tensor.matmul(
                    EEs[:], mm(EsT[:, t, :]), mm(E[:, t, :]),
                    start=False, stop=(t == NT - 1),
                )

            # ---- ET direct build from transposed inputs ----
            etl = sb.tile([P, NT, N], F32)
            ET = sb.tile([P, NT, N], F32)
            for t in range(NT):
                nc.vector.scalar_tensor_tensor(
                    etl[:, t, :], bT8[:, t, :], big, dhT[:, t, :],
                    Alu.mult, Alu.subtract,
                )
                nc.scalar.activation(
                    mm(ET[:, t, :]), etl[:, t, :], Act.Exp, scale=lam
                )

            # ---- W = od ⊙ recip(EEs)  (eps already inside the psum) ----
            zinv = sb.tile([S, N], F32)
            W = sb.tile([S, N], F32)
            nc.vector.reciprocal_approx_fast(zinv[:], EEs[:])
            nc.vector.tensor_mul(mm(W[:]), ods[:], zinv[:])

            # ---- P3(mt) = Es^T @ W; tile 0 also accumulates
            #      T2 = W + W @ E^T into partitions 0..47 (Es == E0[0:48],
            #      so p3_t0 = E0 ⊙ (P3 + pad(T2)) covers the rows terms) ----
            out_t0 = sb.tile([P, N], F16)
            out_t1 = sb.tile([P, N], F16)
            out_t2 = sb.tile([P, N], F16)
            WsT = sb.tile([P, NT, S], F32)
            P30 = psp3.tile([P, N], F32, tag="P30")
            P31 = psp3.tile([P, N], F32, tag="P31")
            P32 = psp3.tile([P, N], F32, tag="P32")
            nc.tensor.matmul(
                P30[:], mm(Es[:, 0:P]), mm(W[:]), start=True, stop=False
            )
            nc.tensor.matmul(
                P30[0:S, :], mm(identm[:]), mm(W[:]), start=False, stop=False
            )
            nc.tensor.matmul(
                P31[:], mm(Es[:, P : 2 * P]), mm(W[:]), start=True, stop=True
            )
            nc.vector.tensor_mul(out_t1[:], E[:, 1, :], P31[:])
            nc.sync.dma_start(p3[:, 1, :], out_t1[:])
            nc.tensor.matmul(
                P32[:], mm(Es[:, 2 * P : N]), mm(W[:]), start=True, stop=True
            )
            nc.vector.tensor_mul(out_t2[:], E[:, 2, :], P32[:])
            nc.sync.dma_start(p3[:, 2, :], out_t2[:])
            for c in range(NT):
                tpw = pst.tile([P, S], F32, tag="tp")
                nc.tensor.transpose(
                    mm(tpw[:]), mm(W[:, P * c : P * (c + 1)]), mm(identm[:])
                )
                nc.scalar.copy(mm(WsT[:, c, :]), tpw[:])
                nc.tensor.matmul(
                    P30[0:S, :], mm(WsT[:, c, :]), mm(ET[:, c, :]),
                    start=False, stop=(c == NT - 1),
                )
            nc.vector.tensor_mul(out_t0[:], E[:, 0, :], P30[:])
            nc.sync.dma_start(p3[:, 0, :], out_t0[:])

    nc.compile()
    return nc


_PROGRAM_CACHE: dict = {}


def _get_program(lam: float) -> bass.Bass:
    if lam not in _PROGRAM_CACHE:
        _PROGRAM_CACHE[lam] = build_program(lam)
    return _PROGRAM_CACHE[lam]


def _tile_rows(x: np.ndarray) -> np.ndarray:
    """[384, N] row-major -> [128, 3, N] partition-tiled layout."""
    return np.ascontiguousarray(x.reshape(NT, P, -1).transpose(1, 0, 2))


def _untile_rows(x: np.ndarray) -> np.ndarray:
    """[128, 3, N] partition-tiled -> [384, N]."""
    return x.transpose(1, 0, 2).reshape(N, -1)


def make_in_maps(od, adj, dist):
    adjz = adj.astype(np.int8)
    np.fill_diagonal(adjz, 0)
    adjz -= 1  # edge -> 0, non-edge -> -1: el = big*(adj-1) - dist
    odz = od.copy()
    np.fill_diagonal(odz, 0.0)
    disth = dist.astype(np.float16)
    in_maps = []
    for i in range(NCORES):
        r = S * i
        a = np.roll(adjz, (-r, -r), axis=(0, 1))
        d = np.roll(disth, (-r, -r), axis=(0, 1))
        ods = np.roll(odz, (-r, -r), axis=(0, 1))[:S]
        in_maps.append(
            {
                "adj8": _tile_rows(a),
                "dsth": _tile_rows(d),
                "adjT8": _tile_rows(np.ascontiguousarray(a.T)),
                "dstTh": _tile_rows(np.ascontiguousarray(d.T)),
                "odt": np.ascontiguousarray(ods),
            }
        )
    return in_maps


def gather(results) -> np.ndarray:
    out = np.zeros((N, N), np.float32)
    for i in range(NCORES):
        r = S * i
        p3f = _untile_rows(results[i]["p3_t"]).astype(np.float32)
        out += np.roll(p3f, (r, r), axis=(0, 1))
    return out


def kernel(od, adj, dist, lambda_param, capacity=None, **_unused) -> np.ndarray:
    od = np.ascontiguousarray(np.asarray(od, dtype=np.float32))
    adj = np.ascontiguousarray(np.asarray(adj, dtype=np.int32))
    dist = np.ascontiguousarray(np.asarray(dist, dtype=np.float32))
    lam = float(np.asarray(lambda_param))
    nc = _get_program(lam)
    res = run_bass_kernel_spmd(nc, make_in_maps(od, adj, dist), list(range(NCORES)))
    return gather(res.results)
